# revision 12
# baseline (speedup 1.0000x reference)
"""LIF spike kernel for Trainium2 (Bass/Tile), data-parallel over batch on 8 cores.

Reparametrized recurrence: with v_t = u_t * 2^t and host-prescaled
x'_t = x_t * 2^t (exact power-of-2 scaling), the LIF step needs no tau
multiply:
  v_t = m'_{t-1} + x'_t ; s_t = v_t > 2^t ; m'_t = (v_t <= 2^t) * v_t

Engine findings baked in: DVE+GpSimd compute thrash each other's SBUF
access (so GpSimd does no tensor ops), DVE+Act coexist cleanly, fp32 DVE
ops are element-rate-bound (~1.22us per [128,1024]). Per (b, t):
  s   = Act sign(v - 2^t) -> i8 slice of a shared per-t [128, 4HW] tile
  m'  = stt(v, 2^t, v, is_le, mult)   DVE, fresh tile
  v'  = tt(m', x'_{t+1}, add)         DVE, fresh tile
  or, for ACCUM slots, v' via software-DGE accum-DMA (m' += x' done by
  the DMA engines; 2x DMA cost, zero DVE cost) to balance the two lanes.

Host layout per core: x_core [C=128, T, B_loc*HW] f32 (prescaled) so each
timestep is ONE [C, 4HW] in-DMA with 16KB/partition descriptor runs and
each timestep's spikes are ONE [C, 4HW] i8 out-DMA (4KB runs). Output
spike decoded as (raw == 1).
"""

import numpy as np

import concourse.bacc as bacc
import concourse.mybir as mybir
from concourse.tile import TileContext
from concourse.bass_utils import run_bass_kernel_spmd

B, T, C, H, W = 32, 8, 128, 32, 32
HW = H * W
N_CORES = 8
B_LOC = B // N_CORES
BHW = B_LOC * HW

f32 = mybir.dt.float32
i8 = mybir.dt.int8
op = mybir.AluOpType
AF = mybir.ActivationFunctionType

# timesteps t (>=1) whose x' arrives via accum-DMA onto m'_{t-1}, per b
ACCUM = [{1}, {1}, {2}, {2}]

_nc_cache = None


def build_nc():
    nc = bacc.Bacc("TRN2", target_bir_lowering=False)
    x = nc.dram_tensor("x", [C, T, BHW], f32, kind="ExternalInput")
    out = nc.dram_tensor("out", [C, T, BHW], i8, kind="ExternalOutput")

    with TileContext(nc) as tc:
        with (
            tc.tile_pool(name="xq", bufs=3) as xq,
            tc.tile_pool(name="vp", bufs=3) as vp,
            tc.tile_pool(name="mp", bufs=2) as mp,
            tc.tile_pool(name="sp_", bufs=3) as spool,
            tc.tile_pool(name="cst", bufs=1) as cst,
        ):
            # Act sign needs bias as a per-partition AP: -2^t for each t
            bias = []
            for t in range(T):
                bt = cst.tile([C, 1], f32, name=f"bias{t}")
                nc.vector.memset(bt[:], -float(2**t))
                bias.append(bt)

            # t=0: per-b fetches so chain b0 starts after ~1.3us
            v_cur = [None] * B_LOC
            for b in range(B_LOC):
                vt = vp.tile([C, HW], f32, tag=f"v{b}", name=f"v0_{b}")
                nc.sync.dma_start(out=vt[:], in_=x[:, 0, b * HW : (b + 1) * HW])
                v_cur[b] = vt

            xt_tiles = [None] * T

            def issue_in(t):
                # just-in-time full-timestep fetch [C, 4HW] (16KB runs);
                # accum-covered (b) slices are fetched anyway (cheap) but
                # unused by those chains
                if 1 <= t < T:
                    xt = xq.tile([C, BHW], f32, tag="x", name=f"x_{t}")
                    nc.sync.dma_start(out=xt[:], in_=x[:, t, :])
                    xt_tiles[t] = xt

            issue_in(1)
            issue_in(2)

            for t in range(T):
                issue_in(t + 3)
                thr = float(2**t)
                st_tile = spool.tile([C, BHW], i8, tag="s", name=f"s_{t}")
                for b in range(B_LOC):
                    # spike output: s = sign(v - 2^t), i8, spike == 1
                    nc.scalar.activation(
                        st_tile[:, b * HW : (b + 1) * HW],
                        v_cur[b][:],
                        AF.Sign,
                        bias=bias[t][:],
                        scale=1.0,
                    )
                for b in range(B_LOC):
                    if t == T - 1:
                        continue
                    v = v_cur[b]
                    # m' = (v <= 2^t) * v ; v' = m' + x'_{t+1}
                    mt = mp.tile([C, HW], f32, tag=f"m{b}", name=f"m_{b}_{t}")
                    nc.vector.scalar_tensor_tensor(
                        mt[:], v[:], thr, v[:], op.is_le, op.mult
                    )
                    if (t + 1) in ACCUM[b]:
                        nc.gpsimd.dma_start(
                            out=mt[:],
                            in_=x[:, t + 1, b * HW : (b + 1) * HW],
                            accum_op=op.add,
                        )
                        v_cur[b] = mt
                    else:
                        vn = vp.tile([C, HW], f32, tag=f"v{b}", name=f"v_{b}_{t}")
                        nc.vector.tensor_tensor(
                            vn[:],
                            mt[:],
                            xt_tiles[t + 1][:, b * HW : (b + 1) * HW],
                            op.add,
                        )
                        v_cur[b] = vn
                # per-t spike out-DMA; split the last two for earlier drain
                if t >= T - 2:
                    for b in range(B_LOC):
                        nc.sync.dma_start(
                            out=out[:, t, b * HW : (b + 1) * HW],
                            in_=st_tile[:, b * HW : (b + 1) * HW],
                        )
                else:
                    nc.sync.dma_start(out=out[:, t, :], in_=st_tile[:])
    nc.compile()
    return nc


def make_in_maps(x: np.ndarray) -> list[dict]:
    xs = np.ascontiguousarray(x).reshape(B, T, C, HW)
    # prescale x'_t = x_t * 2^t (exact in f32)
    scale = (2.0 ** np.arange(T, dtype=np.float32)).astype(np.float32)
    xs = (xs * scale[None, :, None, None]).astype(np.float32)
    return [
        {
            # [b, t, c, hw] -> [c, t, b, hw]
            "x": np.ascontiguousarray(
                xs[i * B_LOC : (i + 1) * B_LOC].transpose(2, 1, 0, 3)
            ).reshape(C, T, BHW)
        }
        for i in range(N_CORES)
    ]


def kernel(x: np.ndarray) -> np.ndarray:
    global _nc_cache
    if _nc_cache is None:
        _nc_cache = build_nc()
    res = run_bass_kernel_spmd(_nc_cache, make_in_maps(x), list(range(N_CORES)))
    # out[c, t, b_loc*HW+hw] -> [b, t, c, hw]; spike iff raw == 1
    parts = [
        (res.results[i]["out"].reshape(C, T, B_LOC, HW) == 1).transpose(2, 1, 0, 3)
        for i in range(N_CORES)
    ]
    full = np.concatenate(parts, axis=0)
    return full.reshape(B, T, C, H, W).astype(np.float32)


# revision 13
# speedup vs baseline: 1.1781x; 1.1781x over previous
"""LIF spike kernel for Trainium2 (Bass/Tile), data-parallel over batch on 8 cores.

Reparametrized recurrence: with v_t = u_t * 2^t and host-prescaled
x'_t = x_t * 2^t (exact power-of-2 scaling), the LIF step needs no tau
multiply:
  v_t = m'_{t-1} + x'_t ; s_t = v_t > 2^t ; m'_t = (v_t <= 2^t) * v_t

Engine findings baked in: DVE+GpSimd thrash each other's SBUF access
(concurrent ops ~3x slower), DVE+Act coexist cleanly, fp32 DVE ops are
element-rate-bound (~1.2us per [128,1024]). So per (b, t):
  s   = Act sign(v - 2^t) -> i8 slice of a paired [128,2048] out tile
  m'  = stt(v, 2^t, v, is_le, mult)   DVE, fresh tile
  v'  = tt(m', x'_{t+1}, add)         DVE, fresh tile
  or, for a few ACCUM slots, v' via software-DGE accum-DMA (m' += x' done
  by the DMA engines; 2x DMA cost but zero DVE cost) to balance lanes.
Host layout per core: x_core [C=128, B_loc=4, T*HW=8192] f32 (prescaled);
output i8 [C, B_loc, T*HW], spike decoded as (raw == 1).
"""

import numpy as np

import concourse.bacc as bacc
import concourse.mybir as mybir
from concourse.tile import TileContext
from concourse.bass_utils import run_bass_kernel_spmd

B, T, C, H, W = 32, 8, 128, 32, 32
HW = H * W
N_CORES = 8
B_LOC = B // N_CORES

f32 = mybir.dt.float32
i8 = mybir.dt.int8
op = mybir.AluOpType
AF = mybir.ActivationFunctionType

# spike-op engine per (b, t): 'a' = Act sign, 'v' = DVE tensor_scalar is_gt
S_ENG = [["a"] * 8 for _ in range(4)]
# timesteps t (>=1) whose x' arrives via accum-DMA onto m'_{t-1}
ACCUM = [{1}, {1}, {2}, {2}]

_nc_cache = None


def build_nc():
    nc = bacc.Bacc("TRN2", target_bir_lowering=False)
    x = nc.dram_tensor("x", [C, B_LOC, T * HW], f32, kind="ExternalInput")
    out = nc.dram_tensor("out", [C, B_LOC, T * HW], i8, kind="ExternalOutput")

    with TileContext(nc) as tc:
        with (
            tc.tile_pool(name="xq", bufs=3) as xq,
            tc.tile_pool(name="vp", bufs=3) as vp,
            tc.tile_pool(name="mp", bufs=2) as mp,
            tc.tile_pool(name="sp_", bufs=3) as spool,
            tc.tile_pool(name="cst", bufs=1) as cst,
        ):
            # Act sign needs bias as a per-partition AP: -2^t for each t
            bias = []
            for t in range(T):
                bt = cst.tile([C, 1], f32, name=f"bias{t}")
                nc.vector.memset(bt[:], -float(2**t))
                bias.append(bt)

            # preload the activation table before data arrives
            warm = cst.tile([C, 1], i8, name="warm")
            nc.scalar.activation(warm[:], bias[0][:], AF.Sign, bias=bias[0][:])

            # t=0 chunks land in the v pool directly (v_0 = x'_0)
            v_cur = [None] * B_LOC
            xt_tiles = [[None] * T for _ in range(B_LOC)]
            for b in range(B_LOC):
                vt = vp.tile([C, HW], f32, tag=f"v{b}")
                nc.sync.dma_start(out=vt[:], in_=x[:, b, 0:HW])
                v_cur[b] = vt

            def issue_in(t):
                # just-in-time x'_t fetches (skipping accum-covered slots) so
                # out-DMAs interleave with in-DMAs on the SP queue
                if 1 <= t < T:
                    for b in range(B_LOC):
                        if t in ACCUM[b]:
                            continue
                        xt = xq.tile([C, HW], f32, tag=f"x{b}")
                        nc.sync.dma_start(
                            out=xt[:], in_=x[:, b, t * HW : (t + 1) * HW]
                        )
                        xt_tiles[b][t] = xt

            issue_in(1)
            issue_in(2)

            s_tiles = [None] * B_LOC
            for t in range(T):
                issue_in(t + 3)
                thr = float(2**t)
                for b in range(B_LOC):
                    v = v_cur[b]
                    # spike output into a paired [C, 2HW] i8 tile (t even:
                    # allocate; t odd: fill second half then DMA out)
                    if t % 2 == 0:
                        s_tiles[b] = spool.tile([C, 2 * HW], i8, tag=f"s{b}", name=f"s{b}_{t}")
                    st = s_tiles[b][:, (t % 2) * HW : (t % 2 + 1) * HW]
                    if S_ENG[b][t] == "a":
                        nc.scalar.activation(
                            st, v[:], AF.Sign, bias=bias[t][:], scale=1.0
                        )
                    else:
                        nc.vector.tensor_scalar(st, v[:], thr, None, op.is_gt)
                    if t == T - 1:
                        nc.sync.dma_start(
                            out=out[:, b, (t - 1) * HW : t * HW],
                            in_=s_tiles[b][:, 0:HW],
                        )
                        nc.sync.dma_start(
                            out=out[:, b, t * HW : (t + 1) * HW],
                            in_=s_tiles[b][:, HW : 2 * HW],
                        )
                    elif t % 2 == 1:
                        nc.sync.dma_start(
                            out=out[:, b, (t - 1) * HW : (t + 1) * HW],
                            in_=s_tiles[b][:],
                        )
                    if t == T - 1:
                        continue
                    # m' = (v <= 2^t) * v ; v' = m' + x'_{t+1}
                    mt = mp.tile([C, HW], f32, tag=f"m{b}")
                    nc.vector.scalar_tensor_tensor(
                        mt[:], v[:], thr, v[:], op.is_le, op.mult
                    )
                    if (t + 1) in ACCUM[b]:
                        nc.gpsimd.dma_start(
                            out=mt[:],
                            in_=x[:, b, (t + 1) * HW : (t + 2) * HW],
                            accum_op=op.add,
                        )
                        v_cur[b] = mt
                    else:
                        vn = vp.tile([C, HW], f32, tag=f"v{b}")
                        nc.vector.tensor_tensor(
                            vn[:], mt[:], xt_tiles[b][t + 1][:], op.add
                        )
                        v_cur[b] = vn
    nc.compile()
    return nc


def make_in_maps(x: np.ndarray) -> list[dict]:
    xs = np.ascontiguousarray(x).reshape(B, T, C, HW)
    # prescale x'_t = x_t * 2^t (exact in f32)
    scale = (2.0 ** np.arange(T, dtype=np.float32)).astype(np.float32)
    xs = (xs * scale[None, :, None, None]).astype(np.float32)
    return [
        {
            "x": np.ascontiguousarray(
                xs[i * B_LOC : (i + 1) * B_LOC].transpose(2, 0, 1, 3)
            ).reshape(C, B_LOC, T * HW)
        }
        for i in range(N_CORES)
    ]


def kernel(x: np.ndarray) -> np.ndarray:
    global _nc_cache
    if _nc_cache is None:
        _nc_cache = build_nc()
    res = run_bass_kernel_spmd(_nc_cache, make_in_maps(x), list(range(N_CORES)))
    # out[c, b_loc, t*HW+hw] -> [b, t, c, hw]; spike iff raw == 1
    parts = [
        (res.results[i]["out"].reshape(C, B_LOC, T, HW) == 1).transpose(1, 2, 0, 3)
        for i in range(N_CORES)
    ]
    full = np.concatenate(parts, axis=0)
    return full.reshape(B, T, C, H, W).astype(np.float32)
